# revision 18
# baseline (speedup 1.0000x reference)
"""Trainium2 Bass kernel for nn_DSVDD (retrieval_knn), fp8 DoubleRow version.

Math (per batch b):
  phi = W @ p_b + bias            [DIM, HW]    (1x1 conv)
  sqdist[i,j] = ||phi_i||^2 + ||C_j||^2 - 2 phi_i . C_j
  top-3 smallest distances d0<=d1<=d2  ->  w0 = 1/(1+exp(d0-d1)+exp(d0-d2))
  score[i] = w0 * d0

Device strategy (8 cores, data-parallel over (batch, HW-half)):
  Both GEMMs run as fp8e4 DoubleRow matmuls (2 MAC/PE/cycle): host quantizes
  p*16, W*1024, 2C*512 to e4m3 (all well under the TRN 240 cap).  The exact
  f32 -8192*||c_j||^2 correction is DVE-added to each psum tile (an ACT
  psum-prewrite + start=False accumulation loses ~1% of the prewrites on
  silicon, and a corr-as-fp8-pair costs 18us of PE), so ys = 8192*Y with
  Y = 2 phi.c - c and no PE cycles go to the correction.  top-3 smallest
  sqdist == top-3 largest Y (f_i common per row).  DVE max8 collects the
  top-8 of each 256..480-wide j-window into runAll; one final max8 per
  i-tile merges the windows.
  f_i = ||phi||^2 comes from DVE squares of the quantized phi (consistent-f)
  reduced by ones-matmuls, deferred one conv step so they never stall the
  PE.  Tail (sqrt, softmin) on ACT/DVE, batched per function so the ACT
  table is not reloaded per i-tile.
"""
import sys

sys.path.insert(0, "/opt/trn_rl_repo")

import numpy as np

B, DIM, H, W_ = 4, 1792, 56, 56
HW = H * W_            # 3136
P = 3136               # prototypes
NCORES = 8
HALF = HW // 2         # 1568 positions per core
KC = DIM // 128        # 14 contraction chunks
KPAIR = KC // 2        # 7 DoubleRow pairs in both GEMMs
IB = 392               # conv i-block (psum width)
NIB = HALF // IB       # 4
PHW = 2 * IB           # 784 p-tile width (stride %16 == 0 for DoubleRow)
JSLICES = [480, 480, 480, 480, 480, 480, 256]   # G-phase j-slices (sum 3136)
# 256 last: the final-merge + sqrt work rides the cheap last window, so the
# post-matmul tail is just exp/softmin + the output DMA.
NJS = len(JSLICES)
NIT = 13               # i-tiles: 12 full + 1 ragged(32)
LAST_W = HALF - 12 * 128   # 32
S_P, S_W, S_PHI, S_C = 16.0, 1024.0, 16.0, 512.0
SYS = S_PHI * S_C      # psum = SYS * (2 phi.c - c)
N_WARM = 20
WT_PREFETCH = 6

_cache = {}


def _build_program():
    import concourse.tile as tile
    from concourse import bacc, mybir

    F32 = mybir.dt.float32
    F32R = mybir.dt.float32r
    FP8 = mybir.dt.float8e4
    AF = mybir.ActivationFunctionType
    ALU = mybir.AluOpType
    AX = mybir.AxisListType
    DR = mybir.MatmulPerfMode.DoubleRow

    nc = bacc.Bacc("TRN2", target_bir_lowering=False, debug=False)

    pT_d = nc.dram_tensor("pT", [DIM, HALF], FP8, kind="ExternalInput")
    wt_d = nc.dram_tensor("wt", [DIM, DIM], FP8, kind="ExternalInput")   # W^T
    cb_d = nc.dram_tensor("cb", [KC * 128, P], FP8, kind="ExternalInput")
    cbc_d = nc.dram_tensor("cbc", [128, P], F32, kind="ExternalInput")
    bias16_d = nc.dram_tensor("bias16", [DIM], F32, kind="ExternalInput")
    onec_d = nc.dram_tensor("onec", [128, 1], F32R, kind="ExternalInput")
    oner_d = nc.dram_tensor("oner", [1, 128], F32R, kind="ExternalInput")
    score_d = nc.dram_tensor("score", [128, NIT], F32, kind="ExternalOutput")

    with tile.TileContext(nc) as tc:
        with (
            tc.tile_pool(name="persist", bufs=1) as persist,
            tc.tile_pool(name="cbp0", bufs=1) as cbp0,
        ):
            phi = persist.tile([128, KC, HALF], FP8)
            bias16_col = persist.tile([128, KC], F32)
            onec = persist.tile([128, 1], F32R)
            oner = persist.tile([1, 128], F32R)
            f_row = persist.tile([1, HALF], F32)
            f_col = persist.tile([128, NIT], F32)
            runAll = persist.tile([128, NIT, NJS, 8], F32)
            runF = persist.tile([128, NIT, 8], F32)
            score_col = persist.tile([128, NIT], F32)

            # ------------- conv phase: phi = W @ p + b, f = ||phi||^2 -------
            with (
                tc.tile_pool(name="pp", bufs=2) as pp,
                tc.tile_pool(name="wtp", bufs=6) as wtp,
                tc.tile_pool(name="sqp", bufs=4) as sqp,
                tc.tile_pool(name="cps", bufs=4, space="PSUM") as cps,
                tc.tile_pool(name="fps", bufs=1, space="PSUM") as fps,
            ):
                f_ps = [fps.tile([1, IB], F32, name=f"fp{ib}", tag=f"f{ib}")
                        for ib in range(NIB)]

                def load_wt(dcg):
                    t = wtp.tile([128, KC, 128], FP8, name="wt_t")
                    nc.sync.dma_start(
                        t[:],
                        wt_d[:, dcg * 128:(dcg + 1) * 128].rearrange(
                            "(cc p) d -> p cc d", p=128),
                    )
                    return t

                def load_p(h):
                    t = pp.tile([128, KC, PHW], FP8, name=f"pq{h}", tag="pq")
                    nc.sync.dma_start(
                        t[:],
                        pT_d[:, h * PHW:(h + 1) * PHW].rearrange(
                            "(cc p) i -> p cc i", p=128),
                    )
                    return t

                # startup-critical loads first: wt chunk 0, then p tile 0
                dcg_seq = list(range(KC)) + list(reversed(range(KC)))  # snake
                wt_tiles = {0: load_wt(dcg_seq[0])}
                wt_issued = 1

                def wt_prefetch(upto):
                    nonlocal wt_issued
                    while wt_issued < min(upto, 2 * KC):
                        if dcg_seq[wt_issued] == dcg_seq[wt_issued - 1]:
                            # snake turn: same chunk again, reuse the tile
                            wt_tiles[wt_issued] = wt_tiles[wt_issued - 1]
                        else:
                            wt_tiles[wt_issued] = load_wt(dcg_seq[wt_issued])
                        wt_issued += 1

                # PE warmup: dummy matmuls keep HAM's activity monitor hot
                # while the first real DMAs land, so conv starts at 2.4 GHz.
                warm = pp.tile([128, 512], F32R, name="warm", tag="warm", bufs=1)
                nc.vector.memset(warm[:].bitcast(F32), 1.0)
                wps = cps.tile([128, 512], F32, name="wps", tag="acc")
                for _ in range(N_WARM):
                    nc.tensor.matmul(wps[:], warm[:, 0:128], warm[:],
                                     start=True, stop=True)

                cb0_t = None
                small_dmas_done = False
                pending_f = []
                for sub in range(2):
                    p_t = load_p(sub)
                    if not small_dmas_done:
                        small_dmas_done = True
                        nc.sync.dma_start(
                            bias16_col[:],
                            bias16_d.rearrange("(g p) -> p g", p=128))
                        nc.sync.dma_start(onec[:], onec_d[:])
                        nc.sync.dma_start(oner[:], oner_d[:])
                    for dcg_i in range(KC):
                        pos = sub * KC + dcg_i
                        dcg = dcg_seq[pos]
                        wt_t = wt_tiles.pop(pos)
                        wt_prefetch(pos + WT_PREFETCH)
                        for k, ib in enumerate((2 * sub, 2 * sub + 1)):
                            if k == 1 and pending_f:
                                # deferred f matmuls: deps long satisfied
                                for args, kw in pending_f:
                                    nc.tensor.matmul(*args, **kw)
                                pending_f = []
                            ioff = (ib % 2) * IB
                            acc = cps.tile([128, IB], F32)
                            for c in range(KPAIR):
                                nc.tensor.matmul(
                                    acc[:],
                                    wt_t[:, 2 * c:2 * c + 2, :],
                                    p_t[:, 2 * c:2 * c + 2, ioff:ioff + IB],
                                    start=(c == 0),
                                    stop=(c == KPAIR - 1),
                                    perf_mode=DR,
                                )
                            isl = slice(ib * IB, (ib + 1) * IB)
                            # phi8 = (psum/1024) + 16 b   (= 16*phi, fp8)
                            nc.scalar.activation(
                                phi[:, dcg, isl], acc[:], AF.Identity,
                                bias=bias16_col[:, dcg:dcg + 1],
                                scale=1.0 / 1024.0,
                            )
                            # sq = phi8^2 = 256*phi^2 (DVE; consistent f)
                            sq = sqp.tile([128, IB], F32R)
                            nc.vector.tensor_tensor(
                                sq[:], phi[:, dcg, isl],
                                phi[:, dcg, isl], ALU.mult,
                            )
                            pending_f.append((
                                (f_ps[ib][:], onec[:], sq[:]),
                                dict(start=(dcg_i == 0), stop=(dcg_i == KC - 1)),
                            ))
                    if sub == 0:
                        # prefetch first G slice (+ its correction) mid-conv
                        j0 = JSLICES[0]
                        cb0_t = cbp0.tile([128, KC, j0], FP8)
                        nc.sync.dma_start(
                            cb0_t[:],
                            cb_d[:, 0:j0].rearrange("(cc p) j -> p cc j",
                                                    p=128),
                        )
                        cbc0_t = cbp0.tile([128, JSLICES[0]], F32, name="cbc0")
                        nc.sync.dma_start(cbc0_t[:], cbc_d[:, 0:j0])
                for args, kw in pending_f:
                    nc.tensor.matmul(*args, **kw)
                pending_f = []
                for ib in range(NIB):
                    nc.vector.tensor_copy(
                        f_row[:, ib * IB:(ib + 1) * IB], f_ps[ib][:]
                    )

            # ------------- f relayout: [1, 1568] -> [128, 13] ---------------
            with tc.tile_pool(name="ftp", bufs=2, space="PSUM") as ftp:
                ft = ftp.tile([128, NIT], F32)
                for it in range(NIT):
                    w = 128 if it < 12 else LAST_W
                    nc.tensor.transpose(
                        ft[0:w, it:it + 1],
                        f_row[:, it * 128:it * 128 + w],
                        oner[0:1, 0:1].bitcast(F32),
                    )
                # f_col = f (sq was 256*phi^2)
                nc.scalar.activation(f_col[:], ft[:], AF.Copy,
                                     scale=1.0 / 256.0)

            # ------------- G phase: psum = 8192*(2 phi.c - c), top-8 --------
            with (
                tc.tile_pool(name="cbp", bufs=2) as cbp,
                tc.tile_pool(name="cbcp", bufs=2) as cbcp,
                tc.tile_pool(name="ysp", bufs=4) as ysp,
                tc.tile_pool(name="tails", bufs=1) as tails,
                tc.tile_pool(name="yps", bufs=8, space="PSUM") as yps,
            ):
                d3s = tails.tile([128, NIT, 3], F32)
                dds = tails.tile([128, NIT, 3], F32)
                ees = tails.tile([128, NIT, 3], F32)
                sss = tails.tile([128, NIT], F32)
                rrs = tails.tile([128, NIT], F32)

                joff = [0]
                for js in range(1, NJS):
                    joff.append(joff[-1] + JSLICES[js - 1])

                for js in range(NJS):
                    w_js = JSLICES[js]
                    jsl = slice(joff[js], joff[js] + w_js)
                    if js == 0:
                        cb_t = cb0_t
                        cbc_t = cbc0_t
                    else:
                        cb_t = cbp.tile([128, KC, w_js], FP8, name="cb_t",
                                        tag="cb")
                        nc.sync.dma_start(
                            cb_t[:],
                            cb_d[:, jsl].rearrange("(cc p) j -> p cc j",
                                                   p=128),
                        )
                        cbc_t = cbcp.tile([128, 512], F32, name="cbc_t",
                                          tag="cbc")
                        nc.sync.dma_start(cbc_t[:, 0:w_js], cbc_d[:, jsl])
                    for it in range(NIT):
                        w = 128 if it < 12 else LAST_W
                        i0 = it * 128
                        y = yps.tile([128, 512], F32, name="y", tag="y")
                        for c in range(KPAIR):
                            nc.tensor.matmul(
                                y[0:w, 0:w_js],
                                phi[:, 2 * c:2 * c + 2, i0:i0 + w],
                                cb_t[:, 2 * c:2 * c + 2, :],
                                start=(c == 0),
                                stop=(c == KPAIR - 1),
                                perf_mode=DR,
                            )
                        # exact f32 -8192*||c_j||^2 correction on DVE
                        ys = ysp.tile([128, 512], F32, name="ys", tag="ys")
                        nc.vector.tensor_tensor(
                            ys[0:w, 0:w_js], y[0:w, 0:w_js],
                            cbc_t[0:w, 0:w_js], ALU.add,
                        )
                        nc.vector.max(runAll[0:w, it, js, :], ys[0:w, 0:w_js])
                        if js == NJS - 1:
                            # merge windows + sqrt while the PE finishes the
                            # remaining i-tiles of this last (cheap) window
                            nc.vector.max(runF[0:w, it, :],
                                          runAll[0:w, it, :, :])
                            nc.scalar.activation(
                                d3s[0:w, it, :], runF[0:w, it, 0:3], AF.Sqrt,
                                bias=f_col[0:w, it:it + 1], scale=-1.0 / SYS,
                            )
                            nc.vector.tensor_scalar(
                                dds[0:w, it, :], d3s[0:w, it, :],
                                d3s[0:w, it, 0:1], None, ALU.subtract,
                            )

                # softmin weight: w0 = 1/sum(exp(-(d_k - d_0)))
                nc.scalar.activation(ees[:], dds[:], AF.Exp, scale=-1.0)
                nc.vector.tensor_reduce(sss[:], ees[:], AX.X, ALU.add)
                nc.vector.reciprocal(rrs[:], sss[:])
                nc.vector.tensor_tensor(score_col[:], d3s[:, :, 0],
                                        rrs[:], ALU.mult)
            nc.sync.dma_start(score_d[:], score_col[:])

    nc.compile()
    return nc


def _get_program():
    if "nc" not in _cache:
        _cache["nc"] = _build_program()
    return _cache["nc"]


def kernel(p, W, b, C):
    import ml_dtypes
    from concourse.bass_utils import run_bass_kernel_spmd

    E4 = ml_dtypes.float8_e4m3

    nc = _get_program()

    p = np.asarray(p, dtype=np.float32)
    W = np.asarray(W, dtype=np.float32)
    b = np.ascontiguousarray(np.asarray(b, dtype=np.float32))
    C = np.asarray(C, dtype=np.float32)

    wt8 = np.ascontiguousarray(W.T * np.float32(S_W)).astype(E4)      # [c, d]
    cn = np.sum(C.astype(np.float64) * C, axis=0)
    cb = np.ascontiguousarray((C * np.float32(2.0 * S_C)).astype(E4))
    cbc = np.ascontiguousarray(np.broadcast_to(
        (-SYS * cn).astype(np.float32)[None, :], (128, P)))
    bias16 = np.ascontiguousarray(b * np.float32(S_PHI))
    onec = np.ones((128, 1), dtype=np.float32)
    oner = np.ones((1, 128), dtype=np.float32)

    p8 = (p.reshape(B, DIM, HW) * np.float32(S_P)).astype(E4)
    in_maps = []
    for core in range(NCORES):
        bidx, half = divmod(core, 2)
        pT = np.ascontiguousarray(p8[bidx, :, half * HALF:(half + 1) * HALF])
        in_maps.append({
            "pT": pT, "wt": wt8, "cb": cb, "cbc": cbc, "bias16": bias16,
            "onec": onec, "oner": oner,
        })

    _cache["last_in_maps"] = in_maps
    res = run_bass_kernel_spmd(nc, in_maps, list(range(NCORES)))
    _cache["last_result"] = res

    return assemble_output(per_core=[res.results[c]["score"] for c in range(NCORES)])


def assemble_output(per_core=None, res_concat=None):
    if per_core is None:
        sc_all = res_concat["score"]                              # [8*128, 13]
        per_core = [sc_all[c * 128:(c + 1) * 128] for c in range(NCORES)]
    out = np.empty((B, 1, H, W_), dtype=np.float32)
    for core in range(NCORES):
        bidx, half = divmod(core, 2)
        sc = per_core[core]                                       # [128, 13]
        flat = np.empty(HALF, dtype=np.float32)
        flat[:12 * 128] = sc[:, :12].T.reshape(-1)
        flat[12 * 128:] = sc[:LAST_W, 12]
        out.reshape(B, 1, HW)[bidx, 0, half * HALF:(half + 1) * HALF] = flat
    return out


# revision 26
# speedup vs baseline: 1.0215x; 1.0215x over previous
"""Trainium2 Bass kernel for nn_DSVDD (retrieval_knn), fp8 DoubleRow version.

Math (per batch b):
  phi = W @ p_b + bias            [DIM, HW]    (1x1 conv)
  sqdist[i,j] = ||phi_i||^2 + ||C_j||^2 - 2 phi_i . C_j
  top-3 smallest distances d0<=d1<=d2  ->  w0 = 1/(1+exp(d0-d1)+exp(d0-d2))
  score[i] = w0 * d0

Device strategy (8 cores, data-parallel over (batch, HW-half)):
  Both GEMMs run as fp8e4 DoubleRow matmuls (2 MAC/PE/cycle): host quantizes
  p*16, W*1024, 2C*512 to e4m3 (all well under the TRN 240 cap).  The exact
  f32 -8192*||c_j||^2 correction is DVE-added to each psum tile (an ACT
  psum-prewrite + start=False accumulation loses ~1% of the prewrites on
  silicon, and a corr-as-fp8-pair costs 18us of PE), so ys = 8192*Y with
  Y = 2 phi.c - c and no PE cycles go to the correction.  top-3 smallest
  sqdist == top-3 largest Y (f_i common per row).  DVE max8 collects the
  top-8 of each 256..480-wide j-window into runAll; one final max8 per
  i-tile merges the windows.
  f_i = ||phi||^2 comes from DVE squares of the quantized phi (consistent-f)
  reduced by ones-matmuls, deferred one conv step so they never stall the
  PE.  Tail (sqrt, softmin) on ACT/DVE, batched per function so the ACT
  table is not reloaded per i-tile.
"""
import sys

sys.path.insert(0, "/opt/trn_rl_repo")

import numpy as np

B, DIM, H, W_ = 4, 1792, 56, 56
HW = H * W_            # 3136
P = 3136               # prototypes
NCORES = 8
HALF = HW // 2         # 1568 positions per core
KC = DIM // 128        # 14 contraction chunks
KPAIR = KC // 2        # 7 DoubleRow pairs in both GEMMs
IB = 392               # conv i-block (psum width)
NIB = HALF // IB       # 4
PHW = 2 * IB           # 784 p-tile width (stride %16 == 0 for DoubleRow)
JSLICES = [480, 480, 480, 480, 480, 480, 256]   # G-phase j-slices (sum 3136)
# 256 last: the final-merge + sqrt work rides the cheap last window, so the
# post-matmul tail is just exp/softmin + the output DMA.
NJS = len(JSLICES)
NIT = 13               # i-tiles: 12 full + 1 ragged(32)
LAST_W = HALF - 12 * 128   # 32
S_P, S_W, S_PHI, S_C = 16.0, 1024.0, 16.0, 512.0
SYS = S_PHI * S_C      # psum = SYS * (2 phi.c - c)
N_WARM = 15
WT_PREFETCH = 4

_cache = {}


def _build_program():
    import concourse.tile as tile
    from concourse import bacc, mybir

    F32 = mybir.dt.float32
    F32R = mybir.dt.float32r
    FP8 = mybir.dt.float8e4
    AF = mybir.ActivationFunctionType
    ALU = mybir.AluOpType
    AX = mybir.AxisListType
    DR = mybir.MatmulPerfMode.DoubleRow

    nc = bacc.Bacc("TRN2", target_bir_lowering=False, debug=False)

    pT_d = nc.dram_tensor("pT", [DIM, HALF], FP8, kind="ExternalInput")
    wt_d = nc.dram_tensor("wt", [DIM, DIM], FP8, kind="ExternalInput")   # W^T
    cb_d = nc.dram_tensor("cb", [KC * 128, P], FP8, kind="ExternalInput")
    cbc_d = nc.dram_tensor("cbc", [128, P], F32, kind="ExternalInput")
    bias16_d = nc.dram_tensor("bias16", [DIM], F32, kind="ExternalInput")
    onec_d = nc.dram_tensor("onec", [128, 1], F32R, kind="ExternalInput")
    oner_d = nc.dram_tensor("oner", [1, 128], F32R, kind="ExternalInput")
    score_d = nc.dram_tensor("score", [128, NIT], F32, kind="ExternalOutput")

    with tile.TileContext(nc) as tc:
        with (
            tc.tile_pool(name="persist", bufs=1) as persist,
            tc.tile_pool(name="cbp0", bufs=1) as cbp0,
        ):
            phi = persist.tile([128, KC, HALF], FP8)
            bias16_col = persist.tile([128, KC], F32)
            onec = persist.tile([128, 1], F32R)
            oner = persist.tile([1, 128], F32R)
            f_row = persist.tile([1, HALF], F32)
            sig = persist.tile([128, NIT], F32)      # sqrt(f)
            rcn = persist.tile([128, NIT], F32)      # 1/sig
            b8p = persist.tile([128, NIT], F32)      # 1/(16384*sig)
            b8n = persist.tile([128, NIT], F32)      # -1/(16384*sig)
            runAll = persist.tile([128, NIT, NJS, 8], F32)
            runF = persist.tile([128, NIT, 8], F32)
            score_col = persist.tile([128, NIT], F32)

            # ------------- conv phase: phi = W @ p + b, f = ||phi||^2 -------
            with (
                tc.tile_pool(name="pp", bufs=2) as pp,
                tc.tile_pool(name="wtp", bufs=6) as wtp,
                tc.tile_pool(name="sqp", bufs=4) as sqp,
                tc.tile_pool(name="cps", bufs=4, space="PSUM") as cps,
                tc.tile_pool(name="fps", bufs=1, space="PSUM") as fps,
            ):
                f_ps = [fps.tile([1, IB], F32, name=f"fp{ib}", tag=f"f{ib}")
                        for ib in range(NIB)]

                def load_wt(dcg):
                    t = wtp.tile([128, KC, 128], FP8, name="wt_t")
                    nc.sync.dma_start(
                        t[:],
                        wt_d[:, dcg * 128:(dcg + 1) * 128].rearrange(
                            "(cc p) d -> p cc d", p=128),
                    )
                    return t

                def load_p(h):
                    t = pp.tile([128, KC, PHW], FP8, name=f"pq{h}", tag="pq")
                    nc.sync.dma_start(
                        t[:],
                        pT_d[:, h * PHW:(h + 1) * PHW].rearrange(
                            "(cc p) i -> p cc i", p=128),
                    )
                    return t

                # startup-critical loads first: wt chunk 0, then p tile 0
                dcg_seq = list(range(KC)) + list(reversed(range(KC)))  # snake
                wt_tiles = {0: load_wt(dcg_seq[0])}
                wt_issued = 1

                def wt_prefetch(upto):
                    nonlocal wt_issued
                    while wt_issued < min(upto, 2 * KC):
                        if dcg_seq[wt_issued] == dcg_seq[wt_issued - 1]:
                            # snake turn: same chunk again, reuse the tile
                            wt_tiles[wt_issued] = wt_tiles[wt_issued - 1]
                        else:
                            wt_tiles[wt_issued] = load_wt(dcg_seq[wt_issued])
                        wt_issued += 1

                # PE warmup: dummy matmuls keep HAM's activity monitor hot
                # while the first real DMAs land, so conv starts at 2.4 GHz.
                warm = pp.tile([128, 512], F32R, name="warm", tag="warm", bufs=1)
                nc.vector.memset(warm[:].bitcast(F32), 1.0)
                wps = cps.tile([128, 512], F32, name="wps", tag="acc")
                for _ in range(N_WARM):
                    nc.tensor.matmul(wps[:], warm[:, 0:128], warm[:],
                                     start=True, stop=True)

                cb0_t = None
                pending_f = []
                p_tiles = [load_p(0), None]
                nc.sync.dma_start(
                    bias16_col[:],
                    bias16_d.rearrange("(g p) -> p g", p=128))
                nc.sync.dma_start(onec[:], onec_d[:])
                nc.sync.dma_start(oner[:], oner_d[:])
                for sub in range(2):
                    p_t = p_tiles[sub]
                    for dcg_i in range(KC):
                        if sub == 0 and dcg_i == 6:
                            # p tile for sub 1: early, before the cb0
                            # prefetch competes for HBM
                            p_tiles[1] = load_p(1)
                        pos = sub * KC + dcg_i
                        dcg = dcg_seq[pos]
                        wt_t = wt_tiles.pop(pos)
                        wt_prefetch(pos + WT_PREFETCH)
                        for k, ib in enumerate((2 * sub, 2 * sub + 1)):
                            if k == 1 and pending_f:
                                # deferred f matmuls: deps long satisfied
                                for args, kw in pending_f:
                                    nc.tensor.matmul(*args, **kw)
                                pending_f = []
                            ioff = (ib % 2) * IB
                            acc = cps.tile([128, IB], F32)
                            for c in range(KPAIR):
                                nc.tensor.matmul(
                                    acc[:],
                                    wt_t[:, 2 * c:2 * c + 2, :],
                                    p_t[:, 2 * c:2 * c + 2, ioff:ioff + IB],
                                    start=(c == 0),
                                    stop=(c == KPAIR - 1),
                                    perf_mode=DR,
                                )
                            isl = slice(ib * IB, (ib + 1) * IB)
                            # phi8 = (psum/1024) + 16 b   (= 16*phi, fp8)
                            nc.scalar.activation(
                                phi[:, dcg, isl], acc[:], AF.Identity,
                                bias=bias16_col[:, dcg:dcg + 1],
                                scale=1.0 / 1024.0,
                            )
                            # sq = phi8^2 = 256*phi^2 (DVE; consistent f)
                            sq = sqp.tile([128, IB], F32R)
                            nc.vector.tensor_tensor(
                                sq[:], phi[:, dcg, isl],
                                phi[:, dcg, isl], ALU.mult,
                            )
                            pending_f.append((
                                (f_ps[ib][:], onec[:], sq[:]),
                                dict(start=(dcg_i == 0), stop=(dcg_i == KC - 1)),
                            ))
                    if sub == 0:
                        # prefetch first G slice (+ its correction) mid-conv
                        j0 = JSLICES[0]
                        cb0_t = cbp0.tile([128, KC, j0], FP8)
                        nc.sync.dma_start(
                            cb0_t[:],
                            cb_d[:, 0:j0].rearrange("(cc p) j -> p cc j",
                                                    p=128),
                        )
                        cbc0_t = cbp0.tile([128, JSLICES[0]], F32, name="cbc0")
                        nc.sync.dma_start(cbc0_t[:], cbc_d[:, 0:j0])
                for args, kw in pending_f:
                    nc.tensor.matmul(*args, **kw)
                pending_f = []
                for ib in range(NIB):
                    nc.vector.tensor_copy(
                        f_row[:, ib * IB:(ib + 1) * IB], f_ps[ib][:]
                    )

            # ------------- f relayout: [1, 1568] -> [128, 13] ---------------
            # sqrt-free tail: with u = Y/8192 << f,
            #   d0 = sqrt(f-u0) ~= sig - u0/(2 sig),  d_k-d_0 ~= (u_k-u0)/(2 sig)
            # so only sig = sqrt(f) is needed (one ACT op; its table load and
            # the Exp table load both hide under the G phase).
            with tc.tile_pool(name="ftp", bufs=2, space="PSUM") as ftp:
                ft = ftp.tile([128, NIT], F32)
                for it in range(NIT):
                    w = 128 if it < 12 else LAST_W
                    nc.tensor.transpose(
                        ft[0:w, it:it + 1],
                        f_row[:, it * 128:it * 128 + w],
                        oner[0:1, 0:1].bitcast(F32),
                    )
                # sig = sqrt(ft/256) (sq was 256*phi^2)
                nc.scalar.activation(sig[:], ft[:], AF.Sqrt,
                                     scale=1.0 / 256.0)
                nc.vector.reciprocal(rcn[:], sig[:])
                nc.vector.tensor_scalar(b8p[:], rcn[:], 1.0 / 16384.0, None,
                                        ALU.mult)
                nc.vector.tensor_scalar(b8n[:], rcn[:], -1.0 / 16384.0, None,
                                        ALU.mult)

            # ------------- G phase: psum = 8192*(2 phi.c - c), top-8 --------
            with (
                tc.tile_pool(name="cbp", bufs=2) as cbp,
                tc.tile_pool(name="cbcp", bufs=2) as cbcp,
                tc.tile_pool(name="ysp", bufs=4) as ysp,
                tc.tile_pool(name="tails", bufs=1) as tails,
                tc.tile_pool(name="yps", bufs=8, space="PSUM") as yps,
            ):
                dYs = tails.tile([128, NIT, 3], F32)
                ees = tails.tile([128, NIT, 3], F32)
                d0c = tails.tile([128, NIT], F32)
                sss = tails.tile([128, NIT], F32)
                rrs = tails.tile([128, NIT], F32)

                joff = [0]
                for js in range(1, NJS):
                    joff.append(joff[-1] + JSLICES[js - 1])

                for js in range(NJS):
                    w_js = JSLICES[js]
                    jsl = slice(joff[js], joff[js] + w_js)
                    if js == 0:
                        cb_t = cb0_t
                        cbc_t = cbc0_t
                    else:
                        cb_t = cbp.tile([128, KC, w_js], FP8, name="cb_t",
                                        tag="cb")
                        nc.sync.dma_start(
                            cb_t[:],
                            cb_d[:, jsl].rearrange("(cc p) j -> p cc j",
                                                   p=128),
                        )
                        cbc_t = cbcp.tile([128, 512], F32, name="cbc_t",
                                          tag="cbc")
                        nc.sync.dma_start(cbc_t[:, 0:w_js], cbc_d[:, jsl])
                    for it in range(NIT):
                        w = 128 if it < 12 else LAST_W
                        i0 = it * 128
                        y = yps.tile([128, 512], F32, name="y", tag="y")
                        for c in range(KPAIR):
                            nc.tensor.matmul(
                                y[0:w, 0:w_js],
                                phi[:, 2 * c:2 * c + 2, i0:i0 + w],
                                cb_t[:, 2 * c:2 * c + 2, :],
                                start=(c == 0),
                                stop=(c == KPAIR - 1),
                                perf_mode=DR,
                            )
                        # exact f32 -8192*||c_j||^2 correction on DVE
                        ys = ysp.tile([128, 512], F32, name="ys", tag="ys")
                        nc.vector.tensor_tensor(
                            ys[0:w, 0:w_js], y[0:w, 0:w_js],
                            cbc_t[0:w, 0:w_js], ALU.add,
                        )
                        nc.vector.max(runAll[0:w, it, js, :], ys[0:w, 0:w_js])
                        if js == NJS - 1:
                            # merge windows + sqrt-free tail pieces while the
                            # PE finishes this last (cheap) window
                            nc.vector.max(runF[0:w, it, :],
                                          runAll[0:w, it, :, :])
                            nc.vector.tensor_scalar(
                                dYs[0:w, it, :], runF[0:w, it, 0:3],
                                runF[0:w, it, 0:1], None, ALU.subtract,
                            )
                            nc.scalar.activation(
                                ees[0:w, it, :], dYs[0:w, it, :], AF.Exp,
                                scale=b8p[0:w, it:it + 1],
                            )
                            nc.vector.tensor_scalar(
                                d0c[0:w, it:it + 1], runF[0:w, it, 0:1],
                                b8n[0:w, it:it + 1], sig[0:w, it:it + 1],
                                ALU.mult, ALU.add,
                            )

                # softmin weight: w0 = 1/sum(exp(-(d_k - d_0)))
                nc.vector.tensor_reduce(sss[:], ees[:], AX.X, ALU.add)
                nc.vector.reciprocal(rrs[:], sss[:])
                nc.vector.tensor_tensor(score_col[:], d0c[:],
                                        rrs[:], ALU.mult)
            nc.sync.dma_start(score_d[:], score_col[:])

    nc.compile()
    return nc


def _get_program():
    if "nc" not in _cache:
        _cache["nc"] = _build_program()
    return _cache["nc"]


def kernel(p, W, b, C):
    import ml_dtypes
    from concourse.bass_utils import run_bass_kernel_spmd

    E4 = ml_dtypes.float8_e4m3

    nc = _get_program()

    p = np.asarray(p, dtype=np.float32)
    W = np.asarray(W, dtype=np.float32)
    b = np.ascontiguousarray(np.asarray(b, dtype=np.float32))
    C = np.asarray(C, dtype=np.float32)

    wt8 = np.ascontiguousarray(W.T * np.float32(S_W)).astype(E4)      # [c, d]
    cn = np.sum(C.astype(np.float64) * C, axis=0)
    cb = np.ascontiguousarray((C * np.float32(2.0 * S_C)).astype(E4))
    cbc = np.ascontiguousarray(np.broadcast_to(
        (-SYS * cn).astype(np.float32)[None, :], (128, P)))
    bias16 = np.ascontiguousarray(b * np.float32(S_PHI))
    onec = np.ones((128, 1), dtype=np.float32)
    oner = np.ones((1, 128), dtype=np.float32)

    p8 = (p.reshape(B, DIM, HW) * np.float32(S_P)).astype(E4)
    in_maps = []
    for core in range(NCORES):
        bidx, half = divmod(core, 2)
        pT = np.ascontiguousarray(p8[bidx, :, half * HALF:(half + 1) * HALF])
        in_maps.append({
            "pT": pT, "wt": wt8, "cb": cb, "cbc": cbc, "bias16": bias16,
            "onec": onec, "oner": oner,
        })

    _cache["last_in_maps"] = in_maps
    res = run_bass_kernel_spmd(nc, in_maps, list(range(NCORES)))
    _cache["last_result"] = res

    return assemble_output(per_core=[res.results[c]["score"] for c in range(NCORES)])


def assemble_output(per_core=None, res_concat=None):
    if per_core is None:
        sc_all = res_concat["score"]                              # [8*128, 13]
        per_core = [sc_all[c * 128:(c + 1) * 128] for c in range(NCORES)]
    out = np.empty((B, 1, H, W_), dtype=np.float32)
    for core in range(NCORES):
        bidx, half = divmod(core, 2)
        sc = per_core[core]                                       # [128, 13]
        flat = np.empty(HALF, dtype=np.float32)
        flat[:12 * 128] = sc[:, :12].T.reshape(-1)
        flat[12 * 128:] = sc[:LAST_W, 12]
        out.reshape(B, 1, HW)[bidx, 0, half * HALF:(half + 1) * HALF] = flat
    return out


# revision 33
# speedup vs baseline: 1.0428x; 1.0209x over previous
"""Trainium2 Bass kernel for nn_DSVDD (retrieval_knn), fp8 DoubleRow version.

Math (per batch b):
  phi = W @ p_b + bias            [DIM, HW]    (1x1 conv)
  sqdist[i,j] = ||phi_i||^2 + ||C_j||^2 - 2 phi_i . C_j
  top-3 smallest distances d0<=d1<=d2  ->  w0 = 1/(1+exp(d0-d1)+exp(d0-d2))
  score[i] = w0 * d0

Device strategy (8 cores, data-parallel over (batch, HW-half)):
  Both GEMMs run as fp8e4 DoubleRow matmuls (2 MAC/PE/cycle): host quantizes
  p*16, W*1024, 2C*512 to e4m3 (all well under the TRN 240 cap).  The exact
  f32 -8192*||c_j||^2 correction is DVE-added to each psum tile (an ACT
  psum-prewrite + start=False accumulation loses ~1% of the prewrites on
  silicon, and a corr-as-fp8-pair costs 18us of PE), so ys = 8192*Y with
  Y = 2 phi.c - c and no PE cycles go to the correction.  top-3 smallest
  sqdist == top-3 largest Y (f_i common per row).  DVE max8 collects the
  top-8 of each 256..480-wide j-window into runAll; one final max8 per
  i-tile merges the windows.
  f_i = ||phi||^2 comes from DVE squares of the quantized phi (consistent-f)
  reduced by ones-matmuls, deferred one conv step so they never stall the
  PE.  Tail (sqrt, softmin) on ACT/DVE, batched per function so the ACT
  table is not reloaded per i-tile.
"""
import sys

sys.path.insert(0, "/opt/trn_rl_repo")

import numpy as np

B, DIM, H, W_ = 4, 1792, 56, 56
HW = H * W_            # 3136
P = 3136               # prototypes
NCORES = 8
HALF = HW // 2         # 1568 positions per core
KC = DIM // 128        # 14 contraction chunks
KPAIR = KC // 2        # 7 DoubleRow pairs in both GEMMs
IB = 392               # conv i-block (psum width)
NIB = HALF // IB       # 4
PHW = 2 * IB           # 784 p-tile width (stride %16 == 0 for DoubleRow)
JSLICES = [480, 480, 480, 480, 480, 480, 256]   # G-phase j-slices (sum 3136)
# 256 last: the final-merge + sqrt work rides the cheap last window, so the
# post-matmul tail is just exp/softmin + the output DMA.
NJS = len(JSLICES)
NIT = 13               # i-tiles: 12 full + 1 ragged(32)
LAST_W = HALF - 12 * 128   # 32
S_P, S_W, S_PHI, S_C = 16.0, 1024.0, 16.0, 512.0
SYS = S_PHI * S_C      # psum = SYS * (2 phi.c - c)
N_WARM = 8
WT_PREFETCH = 4
KPA = 3                # conv p tiles split at pair boundary: planes [0,6) [6,14)

_cache = {}


def _build_program():
    import concourse.tile as tile
    from concourse import bacc, mybir

    F32 = mybir.dt.float32
    F32R = mybir.dt.float32r
    FP8 = mybir.dt.float8e4
    AF = mybir.ActivationFunctionType
    ALU = mybir.AluOpType
    AX = mybir.AxisListType
    DR = mybir.MatmulPerfMode.DoubleRow

    nc = bacc.Bacc("TRN2", target_bir_lowering=False, debug=False)

    pT_d = nc.dram_tensor("pT", [DIM, HALF], FP8, kind="ExternalInput")
    wt_d = nc.dram_tensor("wt", [DIM, DIM], FP8, kind="ExternalInput")   # W^T
    cb_d = nc.dram_tensor("cb", [KC * 128, P], FP8, kind="ExternalInput")
    cbc_d = nc.dram_tensor("cbc", [128, P], F32, kind="ExternalInput")
    bias16_d = nc.dram_tensor("bias16", [DIM], F32, kind="ExternalInput")
    onec_d = nc.dram_tensor("onec", [128, 1], F32R, kind="ExternalInput")
    oner_d = nc.dram_tensor("oner", [1, 128], F32R, kind="ExternalInput")
    score_d = nc.dram_tensor("score", [128, NIT], F32, kind="ExternalOutput")

    with tile.TileContext(nc) as tc:
        with (
            tc.tile_pool(name="persist", bufs=1) as persist,
            tc.tile_pool(name="cbp0", bufs=1) as cbp0,
        ):
            phi = persist.tile([128, KC, HALF], FP8)
            bias16_col = persist.tile([128, KC], F32)
            onec = persist.tile([128, 1], F32R)
            oner = persist.tile([1, 128], F32R)
            f_row = persist.tile([1, HALF], F32)
            sig = persist.tile([128, NIT], F32)      # sqrt(f)
            rcn = persist.tile([128, NIT], F32)      # 1/sig
            b8p = persist.tile([128, NIT], F32)      # 1/(16384*sig)
            b8n = persist.tile([128, NIT], F32)      # -1/(16384*sig)
            runAll = persist.tile([128, NIT, NJS, 8], F32)
            runF = persist.tile([128, NIT, 8], F32)
            score_col = persist.tile([128, NIT], F32)

            # ------------- conv phase: phi = W @ p + b, f = ||phi||^2 -------
            with (
                tc.tile_pool(name="pp", bufs=2) as pp,
                tc.tile_pool(name="wtp", bufs=6) as wtp,
                tc.tile_pool(name="sqp", bufs=4) as sqp,
                tc.tile_pool(name="cps", bufs=4, space="PSUM") as cps,
                tc.tile_pool(name="fps", bufs=1, space="PSUM") as fps,
            ):
                f_ps = [fps.tile([1, IB], F32, name=f"fp{ib}", tag=f"f{ib}")
                        for ib in range(NIB)]

                def load_wt(dcg):
                    t = wtp.tile([128, KC, 128], FP8, name="wt_t")
                    nc.sync.dma_start(
                        t[:],
                        wt_d[:, dcg * 128:(dcg + 1) * 128].rearrange(
                            "(cc p) d -> p cc d", p=128),
                    )
                    return t

                def load_p(h):
                    # two DMAs split at a pair boundary so the first conv
                    # matmuls can start before the whole tile lands
                    ta = pp.tile([128, 2 * KPA, PHW], FP8, name=f"pqa{h}",
                                 tag="pqa")
                    nc.sync.dma_start(
                        ta[:],
                        pT_d[:2 * KPA * 128,
                             h * PHW:(h + 1) * PHW].rearrange(
                            "(cc p) i -> p cc i", p=128),
                    )
                    tb = pp.tile([128, KC - 2 * KPA, PHW], FP8,
                                 name=f"pqb{h}", tag="pqb")
                    nc.sync.dma_start(
                        tb[:],
                        pT_d[2 * KPA * 128:,
                             h * PHW:(h + 1) * PHW].rearrange(
                            "(cc p) i -> p cc i", p=128),
                    )
                    return (ta, tb)

                # startup-critical loads first: wt chunk 0, then p tile 0
                dcg_seq = list(range(KC)) + list(reversed(range(KC)))  # snake
                wt_tiles = {0: load_wt(dcg_seq[0])}
                wt_issued = 1

                def wt_prefetch(upto):
                    nonlocal wt_issued
                    while wt_issued < min(upto, 2 * KC):
                        if dcg_seq[wt_issued] == dcg_seq[wt_issued - 1]:
                            # snake turn: same chunk again, reuse the tile
                            wt_tiles[wt_issued] = wt_tiles[wt_issued - 1]
                        else:
                            wt_tiles[wt_issued] = load_wt(dcg_seq[wt_issued])
                        wt_issued += 1

                # PE warmup: dummy matmuls keep HAM's activity monitor hot
                # while the first real DMAs land, so conv starts at 2.4 GHz.
                warm = pp.tile([128, 512], F32R, name="warm", tag="warm", bufs=1)
                nc.vector.memset(warm[:].bitcast(F32), 1.0)
                wps = cps.tile([128, 512], F32, name="wps", tag="acc")
                for _ in range(N_WARM):
                    nc.tensor.matmul(wps[:], warm[:, 0:128], warm[:],
                                     start=True, stop=True)

                cb0_t = None
                pending_f = []
                p_tiles = [load_p(0), None]
                nc.sync.dma_start(
                    bias16_col[:],
                    bias16_d.rearrange("(g p) -> p g", p=128))
                nc.sync.dma_start(onec[:], onec_d[:])
                nc.sync.dma_start(oner[:], oner_d[:])
                for sub in range(2):
                    p_t = p_tiles[sub]
                    for dcg_i in range(KC):
                        if sub == 0 and dcg_i == 6:
                            # p tile for sub 1: early, before the cb0
                            # prefetch competes for HBM
                            p_tiles[1] = load_p(1)
                        pos = sub * KC + dcg_i
                        dcg = dcg_seq[pos]
                        wt_t = wt_tiles.pop(pos)
                        wt_prefetch(pos + WT_PREFETCH)
                        for k, ib in enumerate((2 * sub, 2 * sub + 1)):
                            if k == 1 and pending_f:
                                # deferred f matmuls: deps long satisfied
                                for args, kw in pending_f:
                                    nc.tensor.matmul(*args, **kw)
                                pending_f = []
                            ioff = (ib % 2) * IB
                            acc = cps.tile([128, IB], F32)
                            for c in range(KPAIR):
                                if c < KPA:
                                    rhs = p_t[0][:, 2 * c:2 * c + 2,
                                                 ioff:ioff + IB]
                                else:
                                    cc = 2 * (c - KPA)
                                    rhs = p_t[1][:, cc:cc + 2, ioff:ioff + IB]
                                nc.tensor.matmul(
                                    acc[:],
                                    wt_t[:, 2 * c:2 * c + 2, :],
                                    rhs,
                                    start=(c == 0),
                                    stop=(c == KPAIR - 1),
                                    perf_mode=DR,
                                )
                            isl = slice(ib * IB, (ib + 1) * IB)
                            # phi8 = (psum/1024) + 16 b   (= 16*phi, fp8)
                            nc.scalar.activation(
                                phi[:, dcg, isl], acc[:], AF.Identity,
                                bias=bias16_col[:, dcg:dcg + 1],
                                scale=1.0 / 1024.0,
                            )
                            # sq = phi8^2 = 256*phi^2 (DVE; consistent f)
                            sq = sqp.tile([128, IB], F32R)
                            nc.vector.tensor_tensor(
                                sq[:], phi[:, dcg, isl],
                                phi[:, dcg, isl], ALU.mult,
                            )
                            pending_f.append((
                                (f_ps[ib][:], onec[:], sq[:]),
                                dict(start=(dcg_i == 0), stop=(dcg_i == KC - 1)),
                            ))
                    if sub == 0:
                        # prefetch first G slice (+ its correction) mid-conv
                        j0 = JSLICES[0]
                        cb0_t = cbp0.tile([128, KC, j0], FP8)
                        nc.sync.dma_start(
                            cb0_t[:],
                            cb_d[:, 0:j0].rearrange("(cc p) j -> p cc j",
                                                    p=128),
                        )
                        cbc0_t = cbp0.tile([128, JSLICES[0]], F32, name="cbc0")
                        nc.sync.dma_start(cbc0_t[:], cbc_d[:, 0:j0])
                for args, kw in pending_f:
                    nc.tensor.matmul(*args, **kw)
                pending_f = []
                for ib in range(NIB):
                    nc.vector.tensor_copy(
                        f_row[:, ib * IB:(ib + 1) * IB], f_ps[ib][:]
                    )

            # ------------- G phase: psum = 8192*(2 phi.c - c), top-8 --------
            # sqrt-free tail: with u = Y/8192 << f,
            #   d0 = sqrt(f-u0) ~= sig - u0/(2 sig),  d_k-d_0 ~= (u_k-u0)/(2 sig)
            # so only sig = sqrt(f) is needed (one ACT op; its table load and
            # the Exp table load both hide under the G phase).  The f
            # relayout (PE transposes) is emitted after the js=0 tiles so the
            # PE never stalls on the f_row copies at the conv->G boundary.
            with (
                tc.tile_pool(name="cbp", bufs=2) as cbp,
                tc.tile_pool(name="cbcp", bufs=2) as cbcp,
                tc.tile_pool(name="ysp", bufs=4) as ysp,
                tc.tile_pool(name="tails", bufs=1) as tails,
                tc.tile_pool(name="yps", bufs=7, space="PSUM") as yps,
            ):
                em = tails.tile([128, 2, NIT], F32)
                em2 = tails.tile([128, 2, NIT], F32)
                ees = tails.tile([128, 2, NIT], F32)
                d0a = tails.tile([128, NIT], F32)
                d0c = tails.tile([128, NIT], F32)
                ss2 = tails.tile([128, NIT], F32)
                sss = tails.tile([128, NIT], F32)
                rrs = tails.tile([128, NIT], F32)

                joff = [0]
                for js in range(1, NJS):
                    joff.append(joff[-1] + JSLICES[js - 1])

                for js in range(NJS):
                    if js == 1:
                        # f relayout [1,1568] -> [128,13]; then sig = sqrt(f)
                        ft = yps.tile([128, NIT], F32, name="ft", tag="ft",
                                      bufs=1)
                        for it in range(NIT):
                            w = 128 if it < 12 else LAST_W
                            nc.tensor.transpose(
                                ft[0:w, it:it + 1],
                                f_row[:, it * 128:it * 128 + w],
                                oner[0:1, 0:1].bitcast(F32),
                            )
                        # sig = sqrt(ft/256) (sq was 256*phi^2)
                        nc.scalar.activation(sig[:], ft[:], AF.Sqrt,
                                             scale=1.0 / 256.0)
                        nc.vector.reciprocal(rcn[:], sig[:])
                        nc.vector.tensor_scalar(b8p[:], rcn[:],
                                                1.0 / 16384.0, None, ALU.mult)
                        nc.vector.tensor_scalar(b8n[:], rcn[:],
                                                -1.0 / 16384.0, None, ALU.mult)
                    w_js = JSLICES[js]
                    jsl = slice(joff[js], joff[js] + w_js)
                    if js == 0:
                        cb_t = cb0_t
                        cbc_t = cbc0_t
                    else:
                        cb_t = cbp.tile([128, KC, w_js], FP8, name="cb_t",
                                        tag="cb")
                        nc.sync.dma_start(
                            cb_t[:],
                            cb_d[:, jsl].rearrange("(cc p) j -> p cc j",
                                                   p=128),
                        )
                        cbc_t = cbcp.tile([128, 512], F32, name="cbc_t",
                                          tag="cbc")
                        nc.sync.dma_start(cbc_t[:, 0:w_js], cbc_d[:, jsl])
                    for it in range(NIT):
                        w = 128 if it < 12 else LAST_W
                        i0 = it * 128
                        y = yps.tile([128, 512], F32, name="y", tag="y")
                        for c in range(KPAIR):
                            nc.tensor.matmul(
                                y[0:w, 0:w_js],
                                phi[:, 2 * c:2 * c + 2, i0:i0 + w],
                                cb_t[:, 2 * c:2 * c + 2, :],
                                start=(c == 0),
                                stop=(c == KPAIR - 1),
                                perf_mode=DR,
                            )
                        # exact f32 -8192*||c_j||^2 correction on DVE
                        ys = ysp.tile([128, 512], F32, name="ys", tag="ys")
                        nc.vector.tensor_tensor(
                            ys[0:w, 0:w_js], y[0:w, 0:w_js],
                            cbc_t[0:w, 0:w_js], ALU.add,
                        )
                        nc.vector.max(runAll[0:w, it, js, :], ys[0:w, 0:w_js])
                        if js == NJS - 1:
                            # merge windows while the PE finishes this last
                            # (cheap) window
                            nc.vector.max(runF[0:w, it, :],
                                          runAll[0:w, it, :, :])

                # batched sqrt-free softmin tail, all [128, NIT]-shaped ops:
                #   em_k  = (Y_k - Y_0) * b8p          (k = 1, 2)
                #   w0    = 1/(1 + exp(em_1) + exp(em_2))
                #   score = w0 * (sig + Y_0 * b8n)
                nc.vector.tensor_tensor(em[:, 0, :], runF[:, :, 1],
                                        runF[:, :, 0], ALU.subtract)
                nc.vector.tensor_tensor(em[:, 1, :], runF[:, :, 2],
                                        runF[:, :, 0], ALU.subtract)
                nc.vector.tensor_tensor(em2[:, 0, :], em[:, 0, :], b8p[:],
                                        ALU.mult)
                nc.vector.tensor_tensor(em2[:, 1, :], em[:, 1, :], b8p[:],
                                        ALU.mult)
                nc.scalar.activation(ees[:], em2[:], AF.Exp)
                nc.vector.tensor_tensor(ss2[:], ees[:, 0, :], ees[:, 1, :],
                                        ALU.add)
                nc.vector.tensor_scalar(sss[:], ss2[:], 1.0, None, ALU.add)
                nc.vector.reciprocal(rrs[:], sss[:])
                nc.vector.tensor_tensor(d0a[:], runF[:, :, 0], b8n[:],
                                        ALU.mult)
                nc.vector.tensor_tensor(d0c[:], d0a[:], sig[:], ALU.add)
                nc.vector.tensor_tensor(score_col[:], d0c[:],
                                        rrs[:], ALU.mult)
            nc.sync.dma_start(score_d[:], score_col[:])

    nc.compile()
    return nc


def _get_program():
    if "nc" not in _cache:
        _cache["nc"] = _build_program()
    return _cache["nc"]


def kernel(p, W, b, C):
    import ml_dtypes
    from concourse.bass_utils import run_bass_kernel_spmd

    E4 = ml_dtypes.float8_e4m3

    nc = _get_program()

    p = np.asarray(p, dtype=np.float32)
    W = np.asarray(W, dtype=np.float32)
    b = np.ascontiguousarray(np.asarray(b, dtype=np.float32))
    C = np.asarray(C, dtype=np.float32)

    wt8 = np.ascontiguousarray(W.T * np.float32(S_W)).astype(E4)      # [c, d]
    cn = np.sum(C.astype(np.float64) * C, axis=0)
    cb = np.ascontiguousarray((C * np.float32(2.0 * S_C)).astype(E4))
    cbc = np.ascontiguousarray(np.broadcast_to(
        (-SYS * cn).astype(np.float32)[None, :], (128, P)))
    bias16 = np.ascontiguousarray(b * np.float32(S_PHI))
    onec = np.ones((128, 1), dtype=np.float32)
    oner = np.ones((1, 128), dtype=np.float32)

    p8 = (p.reshape(B, DIM, HW) * np.float32(S_P)).astype(E4)
    in_maps = []
    for core in range(NCORES):
        bidx, half = divmod(core, 2)
        pT = np.ascontiguousarray(p8[bidx, :, half * HALF:(half + 1) * HALF])
        in_maps.append({
            "pT": pT, "wt": wt8, "cb": cb, "cbc": cbc, "bias16": bias16,
            "onec": onec, "oner": oner,
        })

    _cache["last_in_maps"] = in_maps
    res = run_bass_kernel_spmd(nc, in_maps, list(range(NCORES)))
    _cache["last_result"] = res

    return assemble_output(per_core=[res.results[c]["score"] for c in range(NCORES)])


def assemble_output(per_core=None, res_concat=None):
    if per_core is None:
        sc_all = res_concat["score"]                              # [8*128, 13]
        per_core = [sc_all[c * 128:(c + 1) * 128] for c in range(NCORES)]
    out = np.empty((B, 1, H, W_), dtype=np.float32)
    for core in range(NCORES):
        bidx, half = divmod(core, 2)
        sc = per_core[core]                                       # [128, 13]
        flat = np.empty(HALF, dtype=np.float32)
        flat[:12 * 128] = sc[:, :12].T.reshape(-1)
        flat[12 * 128:] = sc[:LAST_W, 12]
        out.reshape(B, 1, HW)[bidx, 0, half * HALF:(half + 1) * HALF] = flat
    return out


# revision 45
# speedup vs baseline: 1.0477x; 1.0047x over previous
"""Trainium2 Bass kernel for nn_DSVDD (retrieval_knn), fp8 DoubleRow version.

Math (per batch b):
  phi = W @ p_b + bias            [DIM, HW]    (1x1 conv)
  sqdist[i,j] = ||phi_i||^2 + ||C_j||^2 - 2 phi_i . C_j
  top-3 smallest distances d0<=d1<=d2  ->  w0 = 1/(1+exp(d0-d1)+exp(d0-d2))
  score[i] = w0 * d0

Device strategy (8 cores, data-parallel over (batch, HW-half)):
  Both GEMMs run as fp8e4 DoubleRow matmuls (2 MAC/PE/cycle): host quantizes
  p*16, W*1024, 2C*512 to e4m3 (all well under the TRN 240 cap).  The exact
  f32 -8192*||c_j||^2 correction is DVE-added to each psum tile (an ACT
  psum-prewrite + start=False accumulation loses ~1% of the prewrites on
  silicon, and a corr-as-fp8-pair costs 18us of PE), so ys = 8192*Y with
  Y = 2 phi.c - c and no PE cycles go to the correction.  top-3 smallest
  sqdist == top-3 largest Y (f_i common per row).  DVE max8 collects the
  top-8 of each 256..480-wide j-window into runAll; one final max8 per
  i-tile merges the windows.
  f_i = ||phi||^2 comes from DVE squares of the quantized phi (consistent-f)
  reduced by ones-matmuls, deferred one conv step so they never stall the
  PE.  Tail (sqrt, softmin) on ACT/DVE, batched per function so the ACT
  table is not reloaded per i-tile.
"""
import sys

sys.path.insert(0, "/opt/trn_rl_repo")

import numpy as np

B, DIM, H, W_ = 4, 1792, 56, 56
HW = H * W_            # 3136
P = 3136               # prototypes
NCORES = 8
HALF = HW // 2         # 1568 positions per core
KC = DIM // 128        # 14 contraction chunks
KPAIR = KC // 2        # 7 DoubleRow pairs in both GEMMs
KCB = KC + 2           # phi/cb planes incl. the fp8 corr pair (last window)
IB = 392               # conv i-block (psum width)
NIB = HALF // IB       # 4
PHW = 2 * IB           # 784 p-tile width (stride %16 == 0 for DoubleRow)
JSLICES = [480, 480, 480, 480, 480, 480, 256]   # G-phase j-slices (sum 3136)
# 256 last: the final-merge + sqrt work rides the cheap last window, so the
# post-matmul tail is just exp/softmin + the output DMA.
NJS = len(JSLICES)
NIT = 13               # i-tiles: 12 full + 1 ragged(32)
LAST_W = HALF - 12 * 128   # 32
S_P, S_W, S_PHI, S_C = 16.0, 1024.0, 16.0, 512.0
SYS = S_PHI * S_C      # psum = SYS * (2 phi.c - c)
N_WARM = 12
N_WARM2 = 4            # mid-first-group warms covering the p0b DMA wait
WT_PREFETCH = 4
KPA = 3                # conv p tiles split at pair boundary: planes [0,6) [6,14)

_cache = {}


def _build_program():
    import concourse.tile as tile
    from concourse import bacc, mybir

    F32 = mybir.dt.float32
    F32R = mybir.dt.float32r
    FP8 = mybir.dt.float8e4
    AF = mybir.ActivationFunctionType
    ALU = mybir.AluOpType
    AX = mybir.AxisListType
    DR = mybir.MatmulPerfMode.DoubleRow

    nc = bacc.Bacc("TRN2", target_bir_lowering=False, debug=False)

    pT_d = nc.dram_tensor("pT", [DIM, HALF], FP8, kind="ExternalInput")
    wt_d = nc.dram_tensor("wt", [DIM, DIM], FP8, kind="ExternalInput")   # W^T
    cb_d = nc.dram_tensor("cb", [KCB * 128, P], FP8, kind="ExternalInput")
    cbc_d = nc.dram_tensor("cbc", [128, P], F32, kind="ExternalInput")
    bias16_d = nc.dram_tensor("bias16", [DIM], F32, kind="ExternalInput")
    onec_d = nc.dram_tensor("onec", [128, 1], F32R, kind="ExternalInput")
    oner_d = nc.dram_tensor("oner", [1, 128], F32R, kind="ExternalInput")
    score_d = nc.dram_tensor("score", [128, NIT], F32, kind="ExternalOutput")

    with tile.TileContext(nc) as tc:
        with (
            tc.tile_pool(name="persist", bufs=1) as persist,
            tc.tile_pool(name="cbp0", bufs=1) as cbp0,
        ):
            phi = persist.tile([128, KCB, HALF], FP8)
            bias16_col = persist.tile([128, KC], F32)
            onec = persist.tile([128, 1], F32R)
            oner = persist.tile([1, 128], F32R)
            f_row = persist.tile([1, HALF], F32)
            sig = persist.tile([128, NIT], F32)      # sqrt(f)
            rcn = persist.tile([128, NIT], F32)      # 1/sig
            b8p = persist.tile([128, NIT], F32)      # 1/(16384*sig)
            b8n = persist.tile([128, NIT], F32)      # -1/(16384*sig)
            runAll = persist.tile([128, NIT, NJS, 8], F32)
            runF = persist.tile([128, NIT, 8], F32)
            score_col = persist.tile([128, NIT], F32)

            # corr pair planes for the last window: phi[14/15] = 16.0 so that
            # sum_p 16*q8(-2cn_j)*2 = -8192*||c_j||^2 rides the matmul
            nc.gpsimd.memset(phi[:, KC:KCB, :], 16.0)

            # ------------- conv phase: phi = W @ p + b, f = ||phi||^2 -------
            with (
                tc.tile_pool(name="pp", bufs=2) as pp,
                tc.tile_pool(name="wtp", bufs=6) as wtp,
                tc.tile_pool(name="sqp", bufs=4) as sqp,
                tc.tile_pool(name="cps", bufs=4, space="PSUM") as cps,
                tc.tile_pool(name="fps", bufs=1, space="PSUM") as fps,
            ):
                f_ps = [fps.tile([1, IB], F32, name=f"fp{ib}", tag=f"f{ib}")
                        for ib in range(NIB)]

                def load_wt(dcg):
                    t = wtp.tile([128, KC, 128], FP8, name="wt_t")
                    nc.sync.dma_start(
                        t[:],
                        wt_d[:, dcg * 128:(dcg + 1) * 128].rearrange(
                            "(cc p) d -> p cc d", p=128),
                    )
                    return t

                def load_p(h):
                    # two DMAs split at a pair boundary so the first conv
                    # matmuls can start before the whole tile lands
                    ta = pp.tile([128, 2 * KPA, PHW], FP8, name=f"pqa{h}",
                                 tag="pqa")
                    nc.sync.dma_start(
                        ta[:],
                        pT_d[:2 * KPA * 128,
                             h * PHW:(h + 1) * PHW].rearrange(
                            "(cc p) i -> p cc i", p=128),
                    )
                    tb = pp.tile([128, KC - 2 * KPA, PHW], FP8,
                                 name=f"pqb{h}", tag="pqb")
                    nc.sync.dma_start(
                        tb[:],
                        pT_d[2 * KPA * 128:,
                             h * PHW:(h + 1) * PHW].rearrange(
                            "(cc p) i -> p cc i", p=128),
                    )
                    return (ta, tb)

                # startup-critical loads first: wt chunk 0, then p tile 0
                dcg_seq = list(range(KC)) + list(reversed(range(KC)))  # snake
                wt_tiles = {0: load_wt(dcg_seq[0])}
                wt_issued = 1

                def wt_prefetch(upto):
                    nonlocal wt_issued
                    while wt_issued < min(upto, 2 * KC):
                        if dcg_seq[wt_issued] == dcg_seq[wt_issued - 1]:
                            # snake turn: same chunk again, reuse the tile
                            wt_tiles[wt_issued] = wt_tiles[wt_issued - 1]
                        else:
                            wt_tiles[wt_issued] = load_wt(dcg_seq[wt_issued])
                        wt_issued += 1

                # PE warmup: dummy matmuls keep HAM's activity monitor hot
                # while the first real DMAs land, so conv starts at 2.4 GHz.
                warm = pp.tile([128, 512], F32R, name="warm", tag="warm", bufs=1)
                nc.vector.memset(warm[:].bitcast(F32), 1.0)
                wps = cps.tile([128, 512], F32, name="wps", tag="acc")
                for _ in range(N_WARM):
                    nc.tensor.matmul(wps[:], warm[:, 0:128], warm[:],
                                     start=True, stop=True)

                cb0_t = None
                pending_f = []
                p_tiles = [load_p(0), None]
                nc.sync.dma_start(
                    bias16_col[:],
                    bias16_d.rearrange("(g p) -> p g", p=128))
                nc.sync.dma_start(onec[:], onec_d[:])
                nc.sync.dma_start(oner[:], oner_d[:])
                def conv_pair_mm(acc, p_t, wt_t, ioff, c, start, stop):
                    if c < KPA:
                        rhs = p_t[0][:, 2 * c:2 * c + 2, ioff:ioff + IB]
                    else:
                        cc = 2 * (c - KPA)
                        rhs = p_t[1][:, cc:cc + 2, ioff:ioff + IB]
                    nc.tensor.matmul(acc[:], wt_t[:, 2 * c:2 * c + 2, :],
                                     rhs, start=start, stop=stop,
                                     perf_mode=DR)

                for sub in range(2):
                    p_t = p_tiles[sub]
                    accs0 = {}
                    for dcg_i in range(KC):
                        if sub == 0 and dcg_i == 6:
                            # p tile for sub 1: early, before the cb0
                            # prefetch competes for HBM
                            p_tiles[1] = load_p(1)
                        pos = sub * KC + dcg_i
                        dcg = dcg_seq[pos]
                        wt_t = wt_tiles.pop(pos)
                        wt_prefetch(pos + WT_PREFETCH)
                        if sub == 0 and dcg_i == 0:
                            # early pairs (p0a) for both i-blocks, then warms
                            # to cover the p0b DMA, then the late pairs below
                            for ib in (0, 1):
                                accs0[ib] = cps.tile([128, IB], F32,
                                                     name=f"acc0{ib}",
                                                     tag="acc")
                                for c in range(KPA):
                                    conv_pair_mm(accs0[ib], p_t, wt_t,
                                                 (ib % 2) * IB, c,
                                                 c == 0, False)
                            for _ in range(N_WARM2):
                                nc.tensor.matmul(wps[:], warm[:, 0:128],
                                                 warm[:], start=True,
                                                 stop=True)
                        for k, ib in enumerate((2 * sub, 2 * sub + 1)):
                            if k == 1 and pending_f:
                                # deferred f matmuls: deps long satisfied
                                for args, kw in pending_f:
                                    nc.tensor.matmul(*args, **kw)
                                pending_f = []
                            ioff = (ib % 2) * IB
                            if sub == 0 and dcg_i == 0:
                                acc = accs0[ib]
                                crange = range(KPA, KPAIR)
                            else:
                                acc = cps.tile([128, IB], F32)
                                crange = range(KPAIR)
                            for c in crange:
                                conv_pair_mm(acc, p_t, wt_t, ioff, c,
                                             c == 0, c == KPAIR - 1)
                            isl = slice(ib * IB, (ib + 1) * IB)
                            # phi8 = (psum/1024) + 16 b   (= 16*phi, fp8)
                            nc.scalar.activation(
                                phi[:, dcg, isl], acc[:], AF.Identity,
                                bias=bias16_col[:, dcg:dcg + 1],
                                scale=1.0 / 1024.0,
                            )
                            # sq = phi8^2 = 256*phi^2 (DVE; consistent f)
                            sq = sqp.tile([128, IB], F32R)
                            nc.vector.tensor_tensor(
                                sq[:], phi[:, dcg, isl],
                                phi[:, dcg, isl], ALU.mult,
                            )
                            pending_f.append((
                                (f_ps[ib][:], onec[:], sq[:]),
                                dict(start=(dcg_i == 0), stop=(dcg_i == KC - 1)),
                            ))
                    if sub == 0:
                        # prefetch first G slice (+ its correction) mid-conv
                        j0 = JSLICES[0]
                        cb0_t = cbp0.tile([128, KC, j0], FP8)
                        nc.sync.dma_start(
                            cb0_t[:],
                            cb_d[:, 0:j0].rearrange(
                                "(cc p) j -> p cc j", p=128)[:, 0:KC, :],
                        )
                        cbc0_t = cbp0.tile([128, JSLICES[0]], F32, name="cbc0")
                        nc.sync.dma_start(cbc0_t[:], cbc_d[:, 0:j0])
                for args, kw in pending_f:
                    nc.tensor.matmul(*args, **kw)
                pending_f = []
                for ib in range(NIB):
                    nc.vector.tensor_copy(
                        f_row[:, ib * IB:(ib + 1) * IB], f_ps[ib][:]
                    )

            # ------------- G phase: psum = 8192*(2 phi.c - c), top-8 --------
            # sqrt-free tail: with u = Y/8192 << f,
            #   d0 = sqrt(f-u0) ~= sig - u0/(2 sig),  d_k-d_0 ~= (u_k-u0)/(2 sig)
            # so only sig = sqrt(f) is needed (one ACT op; its table load and
            # the Exp table load both hide under the G phase).  The f
            # relayout (PE transposes) is emitted after the js=0 tiles so the
            # PE never stalls on the f_row copies at the conv->G boundary.
            with (
                tc.tile_pool(name="cbp", bufs=2) as cbp,
                tc.tile_pool(name="cbcp", bufs=2) as cbcp,
                tc.tile_pool(name="ysp", bufs=4) as ysp,
                tc.tile_pool(name="tails", bufs=1) as tails,
                tc.tile_pool(name="yps", bufs=7, space="PSUM") as yps,
            ):
                em = tails.tile([128, 2, NIT], F32)
                em2 = tails.tile([128, 2, NIT], F32)
                ees = tails.tile([128, 2, NIT], F32)
                d0a = tails.tile([128, NIT], F32)
                d0c = tails.tile([128, NIT], F32)
                ss2 = tails.tile([128, NIT], F32)
                sss = tails.tile([128, NIT], F32)
                rrs = tails.tile([128, NIT], F32)

                joff = [0]
                for js in range(1, NJS):
                    joff.append(joff[-1] + JSLICES[js - 1])

                for js in range(NJS):
                    if js == 1:
                        # f relayout [1,1568] -> [128,13]; then sig = sqrt(f)
                        ft = yps.tile([128, NIT], F32, name="ft", tag="ft",
                                      bufs=1)
                        for it in range(NIT):
                            w = 128 if it < 12 else LAST_W
                            nc.tensor.transpose(
                                ft[0:w, it:it + 1],
                                f_row[:, it * 128:it * 128 + w],
                                oner[0:1, 0:1].bitcast(F32),
                            )
                        # sig = sqrt(ft/256) (sq was 256*phi^2)
                        nc.scalar.activation(sig[:], ft[:], AF.Sqrt,
                                             scale=1.0 / 256.0)
                        nc.vector.reciprocal(rcn[:], sig[:])
                        nc.vector.tensor_scalar(b8p[:], rcn[:],
                                                1.0 / 16384.0, None, ALU.mult)
                        nc.vector.tensor_scalar(b8n[:], rcn[:],
                                                -1.0 / 16384.0, None, ALU.mult)
                    w_js = JSLICES[js]
                    jsl = slice(joff[js], joff[js] + w_js)
                    last = js == NJS - 1
                    npair = KPAIR + 1 if last else KPAIR
                    if js == 0:
                        cb_t = cb0_t
                        cbc_t = cbc0_t
                    else:
                        # last window carries the fp8 corr pair (planes
                        # 14/15) so no DVE add sits in the final stretch
                        ncc = KCB if last else KC
                        cb_t = cbp.tile([128, ncc, w_js], FP8, name="cb_t",
                                        tag="cbl" if last else "cb")
                        nc.sync.dma_start(
                            cb_t[:],
                            cb_d[:, jsl].rearrange(
                                "(cc p) j -> p cc j", p=128)[:, 0:ncc, :],
                        )
                        if not last:
                            cbc_t = cbcp.tile([128, 512], F32, name="cbc_t",
                                              tag="cbc")
                            nc.sync.dma_start(cbc_t[:, 0:w_js],
                                              cbc_d[:, jsl])
                    for it in range(NIT):
                        w = 128 if it < 12 else LAST_W
                        i0 = it * 128
                        y = yps.tile([128, 512], F32, name="y", tag="y")
                        for c in range(npair):
                            nc.tensor.matmul(
                                y[0:w, 0:w_js],
                                phi[:, 2 * c:2 * c + 2, i0:i0 + w],
                                cb_t[:, 2 * c:2 * c + 2, :],
                                start=(c == 0),
                                stop=(c == npair - 1),
                                perf_mode=DR,
                            )
                        if last:
                            nc.vector.max(runAll[0:w, it, js, :],
                                          y[0:w, 0:w_js])
                            # merge windows while the PE finishes this last
                            # (cheap) window
                            nc.vector.max(runF[0:w, it, :],
                                          runAll[0:w, it, :, :])
                        else:
                            # exact f32 -8192*||c_j||^2 correction on DVE
                            ys = ysp.tile([128, 512], F32, name="ys",
                                          tag="ys")
                            nc.vector.tensor_tensor(
                                ys[0:w, 0:w_js], y[0:w, 0:w_js],
                                cbc_t[0:w, 0:w_js], ALU.add,
                            )
                            nc.vector.max(runAll[0:w, it, js, :],
                                          ys[0:w, 0:w_js])

                # batched sqrt-free softmin tail, all [128, NIT]-shaped ops:
                #   em_k  = (Y_k - Y_0) * b8p          (k = 1, 2)
                #   w0    = 1/(1 + exp(em_1) + exp(em_2))
                #   score = w0 * (sig + Y_0 * b8n)
                nc.vector.tensor_tensor(em[:, 0, :], runF[:, :, 1],
                                        runF[:, :, 0], ALU.subtract)
                nc.vector.tensor_tensor(em[:, 1, :], runF[:, :, 2],
                                        runF[:, :, 0], ALU.subtract)
                nc.vector.tensor_tensor(em2[:, 0, :], em[:, 0, :], b8p[:],
                                        ALU.mult)
                nc.vector.tensor_tensor(em2[:, 1, :], em[:, 1, :], b8p[:],
                                        ALU.mult)
                nc.scalar.activation(ees[:], em2[:], AF.Exp)
                nc.vector.tensor_tensor(ss2[:], ees[:, 0, :], ees[:, 1, :],
                                        ALU.add)
                nc.vector.tensor_scalar(sss[:], ss2[:], 1.0, None, ALU.add)
                nc.vector.reciprocal(rrs[:], sss[:])
                nc.vector.tensor_tensor(d0a[:], runF[:, :, 0], b8n[:],
                                        ALU.mult)
                nc.vector.tensor_tensor(d0c[:], d0a[:], sig[:], ALU.add)
                nc.vector.tensor_tensor(score_col[:], d0c[:],
                                        rrs[:], ALU.mult)
            nc.sync.dma_start(score_d[:], score_col[:])

    nc.compile()
    return nc


def _get_program():
    if "nc" not in _cache:
        _cache["nc"] = _build_program()
    return _cache["nc"]


def kernel(p, W, b, C):
    import ml_dtypes
    from concourse.bass_utils import run_bass_kernel_spmd

    E4 = ml_dtypes.float8_e4m3

    nc = _get_program()

    p = np.asarray(p, dtype=np.float32)
    W = np.asarray(W, dtype=np.float32)
    b = np.ascontiguousarray(np.asarray(b, dtype=np.float32))
    C = np.asarray(C, dtype=np.float32)

    wt8 = np.ascontiguousarray(W.T * np.float32(S_W)).astype(E4)      # [c, d]
    cn = np.sum(C.astype(np.float64) * C, axis=0)
    corr8 = np.asarray(-2.0 * cn, dtype=np.float32).astype(E4)        # [P]
    cb = np.empty((KCB * 128, P), dtype=E4)
    cb[:DIM] = (C * np.float32(2.0 * S_C)).astype(E4)
    cb[DIM:DIM + 128] = corr8[None, :]
    cb[DIM + 128:] = corr8[None, :]
    cb = np.ascontiguousarray(cb)
    cbc = np.ascontiguousarray(np.broadcast_to(
        (-SYS * cn).astype(np.float32)[None, :], (128, P)))
    bias16 = np.ascontiguousarray(b * np.float32(S_PHI))
    onec = np.ones((128, 1), dtype=np.float32)
    oner = np.ones((1, 128), dtype=np.float32)

    p8 = (p.reshape(B, DIM, HW) * np.float32(S_P)).astype(E4)
    in_maps = []
    for core in range(NCORES):
        bidx, half = divmod(core, 2)
        pT = np.ascontiguousarray(p8[bidx, :, half * HALF:(half + 1) * HALF])
        in_maps.append({
            "pT": pT, "wt": wt8, "cb": cb, "cbc": cbc, "bias16": bias16,
            "onec": onec, "oner": oner,
        })

    _cache["last_in_maps"] = in_maps
    res = run_bass_kernel_spmd(nc, in_maps, list(range(NCORES)))
    _cache["last_result"] = res

    return assemble_output(per_core=[res.results[c]["score"] for c in range(NCORES)])


def assemble_output(per_core=None, res_concat=None):
    if per_core is None:
        sc_all = res_concat["score"]                              # [8*128, 13]
        per_core = [sc_all[c * 128:(c + 1) * 128] for c in range(NCORES)]
    out = np.empty((B, 1, H, W_), dtype=np.float32)
    for core in range(NCORES):
        bidx, half = divmod(core, 2)
        sc = per_core[core]                                       # [128, 13]
        flat = np.empty(HALF, dtype=np.float32)
        flat[:12 * 128] = sc[:, :12].T.reshape(-1)
        flat[12 * 128:] = sc[:LAST_W, 12]
        out.reshape(B, 1, HW)[bidx, 0, half * HALF:(half + 1) * HALF] = flat
    return out
